# revision 14
# baseline (speedup 1.0000x reference)
"""AttnBlock (GroupNorm + 4-head d=128 self-attention + residual).

Full input x: [8, 512, 2048] fp32. Data-parallel over batch: core b computes
batch b entirely on-chip (no collectives).

Per-core math (C=512, L=2048, G=4 groups, NH=4 heads, HD=128):
  h  = groupnorm(x)                     fp8e4, [c, l] layout
  q  = wq @ h + bq   [d, l] fp8         (PE-transposed fp8 weights)
  k  = wk @ h + bk   [d, l] fp8
  vT = h^T @ wv^T + bv  [l, d] fp8      (produced transposed; no V transposes)
  sT[k,q] = k_tile^T q   (plain fp8 matmul, fp32 PSUM)
  e = exp(s/sqrt(d) - 2) fp8            (shift cancels in softmax; keeps e
                                         under TRN fp8e4's +-240 max)
  den = ones^T e, avT[d,q] = vT^T e     (DoubleRow fp8 matmuls over k-tile
                                         pairs: K=256 per output column)
  attn = avT * (1/den)  fp8
  out = wo @ attn + bo + x              (DoubleRow over channel-tile pairs)

Measured on TRN2: DoubleRow streams 1 output column/cycle at K=256 (2x the
fp8 FLOPs of a plain matmul), so AV/den/projections run at half the plain
cost; QK (K=128) gains nothing and stays plain fp8. Attention floor per
512-query x 256-key pair: PE = 2xQK + den + av ~ 850ns, ACT = one
[128,2x512] exp ~ 1.1us -> the kernel paces on the scalar engine's exp.

The whole attention stream is software-pipelined GLOBALLY (den/av lag
QK/exp by 2 pairs, crossing chunk boundaries) so exp never waits at a
chunk flush; an exp gap would also drop the PE out of its boosted p-state.
Projection and out-projection work is chopped into ~850ns pieces and
injected one per pair into the early chunks, hiding it under exp.

Prologue: x lands via 16 queue-parallel [128,512] DMAs, weights follow in
q,k,v,o order so the first attention chunk's operands arrive first;
groupnorm stats/solve/apply pipeline per channel-tile behind the DMAs,
split across ACT and DVE. x stays resident in SBUF and serves the
residual. fp8 error lands ~5e-3 rel L2 (the residual dominates the
output, so the attention branch's fp8 noise is scaled down ~25x).

PSUM (8 banks): s pair-slots 2x2banks + den 1 + av 1 + op 2x1.
"""

import os
import numpy as np

import concourse.bass as bass
import concourse.tile as tile
from concourse import bacc, mybir
from concourse.bass_utils import run_bass_kernel_spmd
from concourse.masks import make_identity

F32 = mybir.dt.float32
FP8 = mybir.dt.float8e4

B, C, L = 8, 512, 2048
G = 4            # groupnorm groups; group size 128 == one partition tile
NH, HD = 4, 128  # heads, head dim
CT = C // 128    # 4 channel tiles
LC = L // 512    # 4 l-chunks of 512
LT = L // 128    # 16 l-tiles of 128
NP = LT // 2     # 8 k-tile pairs per softmax row
EPS = 1e-6
SM_SCALE = float(HD) ** -0.5
EXP_SHIFT = -2.0  # exp(logit - 2): cancels in softmax, bounds e in fp8e4
DEPTH = 2         # den/av lag behind QK/exp, in pairs

AFT = mybir.ActivationFunctionType
ALU = mybir.AluOpType
DR = mybir.MatmulPerfMode.DoubleRow


def _register_exp64():
    """Custom DVE op: exp(x*C0*64 + ...) via (C0*x + C1)^64 -- one affine
    stage + 6 squarings (8-stage ALU limit). Lets the vector engine absorb a
    share of softmax exp, which is otherwise scalar-engine-only. Relative
    error vs exp is e^(-z^2/128) for z = logit+shift: <3% over the softmax
    mass, which renormalization mostly cancels (fp8 noise is 3% anyway).
    Registered via the documented dve_ops.OPS extension point."""
    from concourse import dve_ops as DO
    from concourse.dve_spec import Spec, Src0, C0, C1, sq, lower

    for op in DO.OPS:
        if op.name == "EXP64_ATTN":
            return op
    g = Src0 * C0 + C1
    for _ in range(6):
        g = sq(g)
    spec = Spec(
        body=g,
        reference=lambda in0, in1, s0, s1, imm2: (
            (in0.astype(np.float32) * np.float32(s0) + np.float32(s1)) ** 64
        ).astype(np.float32),
    )
    row = max(DO._SUB_OPCODE_FOR_NAME.values()) + 1
    assert row < 0x20, "custom-DVE opcode rows exhausted"
    shas = {}
    for ver in ("v3", "v4"):
        c = DO.DveOpSpec(
            name="EXP64_ATTN", opcode=row, uops=lower(spec, ver=ver),
            rd1_en=DO.has_src1(spec),
        )
        shas[ver] = c.sha(ver)
    op = DO.DveOp("EXP64_ATTN", spec, subdim=False, uops_sha=shas)
    DO.OPS.append(op)
    DO._SUB_OPCODE_FOR_NAME["EXP64_ATTN"] = row
    return op


EXP64 = _register_exp64()
DVE_EXP_PAIRS = (2, 5)  # pairs per chunk whose exp runs on the vector engine


def build_attn_block(nc):
    x_d = nc.dram_tensor("x", [C, L], F32, kind="ExternalInput").ap()
    gs_d = nc.dram_tensor("gn_scale", [C], F32, kind="ExternalInput").ap()
    gb_d = nc.dram_tensor("gn_bias", [C], F32, kind="ExternalInput").ap()
    w_d = {}
    b_d = {}
    for nm in ("q", "k", "v", "o"):
        w_d[nm] = nc.dram_tensor(f"w{nm}", [C, C], F32, kind="ExternalInput").ap()
        b_d[nm] = nc.dram_tensor(f"b{nm}", [C], F32, kind="ExternalInput").ap()
    out_d = nc.dram_tensor("out", [C, L], F32, kind="ExternalOutput").ap()

    with tile.TileContext(nc) as tc:
        with (
            tc.tile_pool(name="const", bufs=1) as const,
            tc.tile_pool(name="wstage", bufs=6) as wstage,
            tc.tile_pool(name="wt", bufs=1) as wt,
            tc.tile_pool(name="big", bufs=1) as big,
            tc.tile_pool(name="small", bufs=4) as small,
            tc.tile_pool(name="epool", bufs=6) as epool,
            tc.tile_pool(name="cpool", bufs=2) as cpool,
            tc.tile_pool(name="psum", bufs=2, space="PSUM") as psum,
        ):
            # ---- constants ----
            identity = const.tile([128, 128], F32)
            make_identity(nc, identity)
            ones = const.tile([128, 128], F32)
            nc.vector.memset(ones, 1.0)
            ones8 = const.tile([128, 2, 128], FP8)
            nc.vector.memset(ones8, 1.0)
            eps_t = const.tile([128, 1], F32)
            nc.vector.memset(eps_t, EPS)
            shift_t = const.tile([128, 1], F32)
            nc.vector.memset(shift_t, EXP_SHIFT)

            def load_cvec(name, ap_1d):
                t = const.tile([128, CT], F32, name=name)
                nc.sync.dma_start(out=t, in_=ap_1d.rearrange("(t p) -> p t", p=128))
                return t

            # gn scale/bias first: the stats solve chain needs them earliest
            gs_sb = load_cvec("gs_sb", gs_d)
            gb_sb = load_cvec("gb_sb", gb_d)

            # ---- x: 16 queue-parallel DMAs; stats follow per piece ----
            x_r = x_d.rearrange("(t p) l -> p t l", p=128)
            x_sb = big.tile([128, CT, L], F32, tag="x_sb")
            h_sb = big.tile([128, CT, L], FP8, tag="h_sb")
            stats_t = []
            for ct in range(CT):
                st = small.tile([128, 4, 6], F32, tag="stats", bufs=CT, name="st")
                stats_t.append(st)
                for p in range(8):
                    nc.sync.dma_start(
                        out=x_sb[:, ct, p * 256 : (p + 1) * 256],
                        in_=x_r[:, ct, p * 256 : (p + 1) * 256],
                    )

            # remaining biases follow x on the queues
            bq_sb = load_cvec("bq_sb", b_d["q"])
            bk_sb = load_cvec("bk_sb", b_d["k"])
            bo_sb = load_cvec("bo_sb", b_d["o"])
            bv_bc = const.tile([128, 2, C], F32, name="bv_bc")  # bv broadcast
            nc.sync.dma_start(
                out=bv_bc,
                in_=bass.AP(
                    tensor=b_d["v"].tensor,
                    offset=b_d["v"].offset,
                    ap=[[0, 128], [0, 2]] + list(b_d["v"].ap),
                ),
            )

            # ---- weights: DMA row-blocks + PE-transpose (fp32) into
            #      wT[c, o], converting to fp8e4 in the PSUM->SBUF drain.
            #      q,k first: attention head 0 needs them before v,o. ----
            wts = {}
            for nm in ("q", "k", "v", "o"):
                wts[nm] = wt.tile([128, CT, C], FP8, name=f"w{nm}t")
            wblocks = [(nm, ot) for nm in ("q", "k", "v", "o") for ot in range(CT)]
            wbi = [0]

            def emit_weight_blocks(n):
                for _ in range(n):
                    if wbi[0] >= len(wblocks):
                        return
                    nm, ot = wblocks[wbi[0]]
                    wbi[0] += 1
                    stg = wstage.tile([128, C], F32, tag="stg")
                    for hf in range(2):
                        nc.sync.dma_start(
                            out=stg[:, hf * 256 : (hf + 1) * 256],
                            in_=w_d[nm][
                                ot * 128 : (ot + 1) * 128, hf * 256 : (hf + 1) * 256
                            ],
                        )
                    pt = psum.tile([128, 4, 128], F32, tag="av", bufs=2, name="pt")
                    for ct in range(CT):
                        nc.tensor.transpose(
                            pt[:, ct, :],
                            stg[:, ct * 128 : (ct + 1) * 128],
                            identity,
                        )
                    dstw = wts[nm][:, :, ot * 128 : (ot + 1) * 128]
                    if wbi[0] % 2 == 0:
                        nc.scalar.copy(dstw, pt)
                    else:
                        nc.vector.tensor_copy(dstw, pt)

            # ---- groupnorm stats solve + apply, pipelined per ct ----
            for ct in range(CT):
                stats = stats_t[ct]
                for p in range(4):
                    nc.vector.bn_stats(
                        out=stats[:, p, :],
                        in_=x_sb[:, ct, p * 512 : (p + 1) * 512],
                    )
                mv = small.tile([128, 2], F32, tag="mv")
                nc.vector.bn_aggr(out=mv, in_=stats)
                # stat2 = [mean_p, E[x^2]_p]
                stat2 = small.tile([128, 2], F32, tag="stat2")
                nc.vector.tensor_copy(stat2[:, 0:1], mv[:, 0:1])
                nc.vector.scalar_tensor_tensor(
                    out=stat2[:, 1:2],
                    in0=mv[:, 0:1],
                    scalar=mv[:, 0:1],
                    in1=mv[:, 1:2],
                    op0=ALU.mult,
                    op1=ALU.add,
                )
                pg = psum.tile([128, 2], F32, tag="den", bufs=1)
                nc.tensor.matmul(pg, ones, stat2, start=True, stop=True)
                mean_t = small.tile([128, 1], F32, tag="mean_t")
                nc.vector.tensor_scalar_mul(mean_t, pg[:, 0:1], 1.0 / 128.0)
                ex2_t = small.tile([128, 1], F32, tag="ex2_t")
                nc.vector.tensor_scalar_mul(ex2_t, pg[:, 1:2], 1.0 / 128.0)
                var_t = small.tile([128, 1], F32, tag="var_t")
                nc.vector.tensor_mul(var_t, mean_t, mean_t)
                nc.vector.tensor_sub(var_t, ex2_t, var_t)
                std_t = small.tile([128, 1], F32, tag="std_t")
                nc.scalar.activation(std_t, var_t, AFT.Sqrt, bias=eps_t)
                rstd_t = small.tile([128, 1], F32, tag="rstd_t")
                nc.vector.reciprocal(rstd_t, std_t)
                a_t = small.tile([128, 1], F32, tag="a_t", bufs=CT)
                nc.vector.tensor_mul(a_t, rstd_t, gs_sb[:, ct : ct + 1])
                b_t = small.tile([128, 1], F32, tag="b_t", bufs=CT)
                nc.vector.tensor_mul(b_t, mean_t, a_t)
                nc.vector.tensor_sub(b_t, gb_sb[:, ct : ct + 1], b_t)
                # apply: h = a*x + b (fp8 out), ct 0-1 on ACT, 2-3 on DVE
                for l2 in range(2):
                    if ct < 2:
                        nc.scalar.activation(
                            h_sb[:, ct, l2 * 1024 : (l2 + 1) * 1024],
                            x_sb[:, ct, l2 * 1024 : (l2 + 1) * 1024],
                            AFT.Identity,
                            bias=b_t,
                            scale=a_t,
                        )
                    else:
                        nc.vector.tensor_scalar(
                            out=h_sb[:, ct, l2 * 1024 : (l2 + 1) * 1024],
                            in0=x_sb[:, ct, l2 * 1024 : (l2 + 1) * 1024],
                            scalar1=a_t,
                            scalar2=b_t,
                            op0=ALU.mult,
                            op1=ALU.add,
                        )
                emit_weight_blocks(4)

            # ---- projections (DoubleRow over channel-tile pairs) ----
            q_sb = big.tile([128, NH, LC, 512], FP8, tag="q_sb")
            k_sb = big.tile([128, NH, LC, 512], FP8, tag="k_sb")
            vT_sb = big.tile([128, LT, C], FP8, tag="vT_sb")

            def emit_proj_piece(dst, wtt, bias, h, lc2):
                # one [128, 2x512] psum worth of q or k: 4 DR matmuls + drain
                pp = psum.tile([128, 2, 512], F32, tag="s", name="pp")
                for j in range(2):
                    lc = 2 * lc2 + j
                    for t in range(CT // 2):
                        nc.tensor.matmul(
                            pp[:, j, :],
                            wtt[:, 2 * t : 2 * t + 2, h * 128 : (h + 1) * 128],
                            h_sb[:, 2 * t : 2 * t + 2, lc * 512 : (lc + 1) * 512],
                            start=(t == 0),
                            stop=(t == CT // 2 - 1),
                            perf_mode=DR,
                        )
                nc.vector.tensor_scalar_add(
                    dst[:, h, 2 * lc2 : 2 * lc2 + 2, :], pp, bias[:, h : h + 1]
                )

            def emit_qk_proj(h):
                for dst, wtt, bias in (
                    (q_sb, wts["q"], bq_sb),
                    (k_sb, wts["k"], bk_sb),
                ):
                    for lc2 in range(LC // 2):
                        emit_proj_piece(dst, wtt, bias, h, lc2)

            def emit_v_proj(m):
                # two vT l-tiles per psum tile: vT[l=128, C] = h_pair^T @ wvT
                pp = psum.tile([128, 2, 512], F32, tag="s", name="pv")
                for j in range(2):
                    lt = 2 * m + j
                    for t in range(CT // 2):
                        nc.tensor.matmul(
                            pp[:, j, :],
                            h_sb[:, 2 * t : 2 * t + 2, lt * 128 : (lt + 1) * 128],
                            wts["v"][:, 2 * t : 2 * t + 2, :],
                            start=(t == 0),
                            stop=(t == CT // 2 - 1),
                            perf_mode=DR,
                        )
                nc.vector.tensor_add(vT_sb[:, 2 * m : 2 * m + 2, :], pp, bv_bc)

            emit_qk_proj(0)
            for m in range(4):
                emit_v_proj(m)

            # ---- attention: one global software pipeline over all 128
            #      (qc, h, pair) steps; den/av + finishes lag DEPTH pairs ----
            attn_sb = big.tile([128, NH, L], FP8, tag="attn_sb")

            def emit_qk_pair(h, qc, t):
                ps = psum.tile([128, 2, 512], F32, tag="s", name="ps")
                for j in range(2):
                    kt = 2 * t + j
                    nc.tensor.matmul(
                        ps[:, j, :],
                        k_sb[:, h, kt // 4, (kt % 4) * 128 : (kt % 4) * 128 + 128],
                        q_sb[:, h, qc, :],
                        start=True,
                        stop=True,
                    )
                return ps

            def emit_out_piece(lc, ot):
                pp = psum.tile([128, 512], F32, tag="op", bufs=1, name="po")
                for t in range(CT // 2):
                    nc.tensor.matmul(
                        pp,
                        wts["o"][:, 2 * t : 2 * t + 2, ot * 128 : (ot + 1) * 128],
                        attn_sb[:, 2 * t : 2 * t + 2, lc * 512 : (lc + 1) * 512],
                        start=(t == 0),
                        stop=(t == CT // 2 - 1),
                        perf_mode=DR,
                    )
                ot_sb = cpool.tile([128, 512], F32, tag="ot_sb")
                nc.vector.scalar_tensor_tensor(
                    out=ot_sb,
                    in0=pp,
                    scalar=bo_sb[:, ot : ot + 1],
                    in1=x_sb[:, ot, lc * 512 : (lc + 1) * 512],
                    op0=ALU.add,
                    op1=ALU.add,
                )
                nc.sync.dma_start(
                    out=out_d[ot * 128 : (ot + 1) * 128, lc * 512 : (lc + 1) * 512],
                    in_=ot_sb,
                )

            # PE-work injections (~850ns pieces), keyed by global pair index:
            # remaining v tiles and heads 1-3 q/k projections ride the early
            # chunks; each out-projection rides the chunk after its qc ends.
            inject = {}
            for i, m in enumerate(range(4, 8)):      # vT l-tiles 8..15
                inject.setdefault(1 + i, []).append(lambda m=m: emit_v_proj(m))
            pieces = [
                (dst, wtt, bias, h, lc2)
                for h in (1, 2, 3)
                for dst, wtt, bias in (
                    (q_sb, wts["q"], bq_sb),
                    (k_sb, wts["k"], bk_sb),
                )
                for lc2 in range(LC // 2)
            ]
            piece_slots = [5, 5, 6, 6, 10, 11, 12, 13, 18, 19, 20, 21]
            for slot, (dst, wtt, bias, h, lc2) in zip(piece_slots, pieces):
                inject.setdefault(slot, []).append(
                    lambda dst=dst, wtt=wtt, bias=bias, h=h, lc2=lc2:
                        emit_proj_piece(dst, wtt, bias, h, lc2)
                )
            for qc in range(LC - 1):
                base = (qc + 1) * NH * NP  # start of chunk (qc+1, h0)
                for ot in range(CT):
                    inject.setdefault(base + 2 + ot, []).append(
                        lambda qc=qc, ot=ot: emit_out_piece(qc, ot)
                    )

            live = {}
            pipe = []  # (qc, h, t, e2)

            def drain_one():
                qc, h, t, e2 = pipe.pop(0)
                if t == 0:
                    live[(qc, h)] = (
                        psum.tile([128, 512], F32, tag="den", bufs=1, name="pden"),
                        psum.tile([128, 512], F32, tag="av", bufs=2, name="pav"),
                    )
                pden, pav = live[(qc, h)]
                nc.tensor.matmul(
                    pden, ones8, e2,
                    start=(t == 0), stop=(t == NP - 1), perf_mode=DR,
                )
                nc.tensor.matmul(
                    pav,
                    vT_sb[:, 2 * t : 2 * t + 2, h * 128 : (h + 1) * 128],
                    e2,
                    start=(t == 0), stop=(t == NP - 1), perf_mode=DR,
                )
                if t == NP - 1:
                    rden = cpool.tile([128, 512], F32, tag="rden", name="rden")
                    nc.vector.reciprocal_approx_fast(rden, pden)
                    nc.vector.tensor_mul(
                        attn_sb[:, h, qc * 512 : (qc + 1) * 512], pav, rden
                    )
                    del live[(qc, h)]

            gi = 0
            for qc in range(LC):
                for h in range(NH):
                    for t in range(NP):
                        ps = emit_qk_pair(h, qc, t)
                        e2 = epool.tile([128, 2, 512], FP8, tag="e2", bufs=6)
                        if t in DVE_EXP_PAIRS:
                            nc.vector._custom_dve(
                                EXP64, out=e2, in0=ps,
                                s0=SM_SCALE / 64.0, s1=1.0 + EXP_SHIFT / 64.0,
                            )
                        else:
                            nc.scalar.activation(
                                e2, ps, AFT.Exp, scale=SM_SCALE, bias=shift_t
                            )
                        pipe.append((qc, h, t, e2))
                        if len(pipe) > DEPTH:
                            drain_one()
                        for fn in inject.pop(gi, ()):
                            fn()
                        gi += 1
            while pipe:
                drain_one()
            for ot in range(CT):
                emit_out_piece(LC - 1, ot)
    nc.compile()
    return nc


_NC_CACHE = {}


def _get_nc():
    if "nc" not in _NC_CACHE:
        nc = bacc.Bacc("TRN2", debug=False)
        build_attn_block(nc)
        _NC_CACHE["nc"] = nc
    return _NC_CACHE["nc"]


def run(trace=False, **inputs):
    nc = _get_nc()
    xs = np.ascontiguousarray(np.asarray(inputs["x"], dtype=np.float32))
    shared = {}
    for nm in ("gn_scale", "gn_bias", "wq", "bq", "wk", "bk", "wv", "bv", "wo", "bo"):
        shared[nm] = np.ascontiguousarray(np.asarray(inputs[nm], dtype=np.float32))
    in_maps = [dict(shared, x=xs[b]) for b in range(B)]
    res = run_bass_kernel_spmd(nc, in_maps, core_ids=list(range(B)), trace=trace)
    out = np.stack([res.results[b]["out"] for b in range(B)], axis=0)
    return out, res


def kernel(**inputs):
    out, _ = run(trace=bool(os.environ.get("ATTN_TRACE")), **inputs)
    return out
